# revision 45
# baseline (speedup 1.0000x reference)
"""Self-contained Trainium2 Bass kernel for single-head full-dim attention.

Reference computation (fp32 jax):
    q  = x @ Wq                      # [B, Nq, D]
    kv = y @ Wkv                     # [B, Nkv, 2D] -> k, v
    attn = softmax(q * D^-0.5 @ k^T) # [B, Nq, Nkv]
    out  = attn @ v                  # [B, Nq, D]
with B=4, Nq=Nkv=2048, D=1024.

Distribution: data parallel over 8 NeuronCores, shard = (batch b,
kv-half s).  Each core computes the UNNORMALIZED output block
out'_s = exp(S_s) @ v_s for its 1024 keys plus the partial softmax
denominator Z_s; the host combines the halves:
out = (out'_0 + out'_1) / (Z_0 + Z_1).  No collectives.

Algebraic fold + reassociation: scores = x @ (scale*Wq@Wk^T) @ y^T.
The host precomputes W* = scale*Wq@Wk^T once (fp32, free), and the
device associates the product as  x @ (W* @ y^T):
    P2:  u_s = W* @ y_s^T        [D, 1024]   (kv-SHARDED -> no dup!)
    P4:  scoresT_s = u_s^T-contracted with x (via lhsT=u tiles)
Compared to the t = x@W* association (which every core must compute
for ALL 2048 queries, duplicated across the kv-pair), u is per-shard
by construction: per-core MACs drop 7.52e9 -> 6.45e9, the theoretical
no-duplication floor, with zero communication.

Layout trick: everything on-chip is computed transposed
([feature, token]) so the TensorEngine contracts along partitions
without any on-chip transposes.  Matmul operands are bf16 (fp32 PSUM
accumulation), EXCEPT kv-tiles 0-1 of P7's contraction, which are fp8
e4m3 in DoubleRow-interleaved [128, ko=2, n] layout: one double-pumped
DoubleRow matmul replaces two bf16 matmuls per output group (-5.7us).
Partial fp8 (2 of 8 kv-tiles) adds sqrt(1/4)*2.6e-2 ~ 1.3e-2 error;
total 1.40e-2 vs the 2e-2 gate.  All exp tiles carry a global 0.5
scale (activation bias=-ln2) so fp8 conversion cannot overflow (raw
exp reaches ~244 > fp8e4's 240 max -> Inf); the scale cancels in
out = num/Z.  Softmax uses exp without max-subtraction (scores ~
N(0,1) by construction); Z is a bf16 vector-engine add-tree plus 4
bf16 ones-matmuls (f32 ones-matmuls run 2-pass LOW_HIGH on the PE).
Outputs are bf16 (host combines in f32) which halves writeback DMA.

DMA: all input DMAs on the sync queue in consumption order (yT, wv,
ws, xT).  A lone queue sustains ~166GB/s; spreading across engines
makes each queue drop to ~100GB/s (shared SDMA engine pool) and slows
the head-critical yT -- measured, not theorized.
"""

import numpy as np
import ml_dtypes

import concourse.bass as bass
import concourse.mybir as mybir
import concourse.tile as tile
from concourse.bass import ds
from concourse.bass_utils import run_bass_kernel_spmd

DIM = 1024
B = 4
NQ = 2048
NKV = 2048
N_CORES = 8
NKV_SHARD = 1024  # keys per core

BF16 = mybir.dt.bfloat16
F32 = mybir.dt.float32
FP8 = mybir.dt.float8e4
LN2 = 0.6931471805599453
NP_BF16 = ml_dtypes.bfloat16

N_WARM = 20


def _split_sync_waits(nc, max_waits: int = 1):
    """walrus in this toolchain rejects instructions carrying more than one
    sem wait ("Too many sync wait commands").  Hoist extra waits onto
    preceding same-engine NOPs: the engine dispatches in order, so waiting
    just before the instruction is semantically identical (at worst it
    delays issue slightly)."""
    import bass_rust as _bass_rust

    for f in nc.m.functions:
        for bb in f.blocks:
            insts = list(bb.instructions)
            out = []
            changed = False
            for inst in insts:
                si = getattr(inst, "sync_info", None)
                waits = list(si.on_wait) if si is not None and si.on_wait else []
                if len(waits) > max_waits:
                    changed = True
                    extra, keep = waits[:-max_waits], waits[-max_waits:]
                    for k in range(0, len(extra), max_waits):
                        nop = mybir.InstNoOp(
                            name=f"{inst.name}_sw{k}", engine=inst.engine,
                            ins=[], outs=[],
                        )
                        nop.sync_info = _bass_rust.SyncInfo(
                            on_wait=extra[k : k + max_waits], on_update=[]
                        )
                        out.append(nop)
                    si.on_wait = keep
                    inst.sync_info = si
                out.append(inst)
            if changed:
                bb.instructions = out


def _early_sp_start(nc):
    """Release the SP (sync) engine from the TileContext entry barrier in
    the `main` block so input dma_starts issue ~2.3us earlier.

    The entry barrier is held hostage by the PE engine's wait on the
    runtime start event ($E[4], ~3us).  SP's input DMAs have no real
    dependency on the other engines: every consumer of a DMA'd tile
    waits on that DMA's completion semaphore anyway.  Protocol-preserving
    edit: SP's drain keeps its arrival increment (gather still sees 4
    arrivals) but loses its wait; SP's barrier event-sem is emptied (no
    wait, no release consumption); the Pool release add drops 4 -> 3 to
    match the remaining three consumers."""
    import bass_rust as _bass_rust

    main = nc.m.functions[0].blocks[0]
    assert main.name == "main", main.name
    for inst in main.instructions:
        eng = str(inst.engine)
        tn = type(inst).__name__
        si = getattr(inst, "sync_info", None)
        if si is None:
            continue
        waits = list(si.on_wait or [])
        upds = list(si.on_update or [])
        is_barrier_wait = any(
            getattr(w, "ant_name", "").startswith("barrier_") for w in waits
        )
        if eng.endswith("SP") and tn == "InstDrain" and is_barrier_wait:
            inst.sync_info = _bass_rust.SyncInfo(on_wait=[], on_update=upds)
        elif eng.endswith("SP") and tn == "InstEventSemaphore" and is_barrier_wait:
            inst.sync_info = _bass_rust.SyncInfo(on_wait=[], on_update=[])
        elif eng.endswith("Pool") and tn == "InstEventSemaphore" and not waits:
            # the release: add-imm 4 -> 3
            new_upds = []
            for u in upds:
                if u.update_mode == "sem-add-imm" and u.update_value == 4:
                    u = _bass_rust.SyncUpdate(
                        sync_type=u.sync_type, id=u.id, ant_name=u.ant_name,
                        update_mode=u.update_mode, update_value=3,
                        update_reg=u.update_reg,
                    )
                new_upds.append(u)
            inst.sync_info = _bass_rust.SyncInfo(on_wait=[], on_update=new_upds)


def build_attention_nc():
    """Build the per-core Bass graph (identical on all 8 cores)."""
    nc = bass.Bass(enable_partition_id=False)

    # DRAM parameters (per-core shards, host-prepped layouts; all bf16).
    xT_d = nc.declare_dram_parameter("xT", [DIM, NQ], BF16, isOutput=False)
    yT_d = nc.declare_dram_parameter("yT", [DIM, NKV_SHARD], BF16, isOutput=False)
    # W*^T slabs pre-arranged so each DMA is per-partition contiguous:
    # ws[j, p, c, m] = W*T[c*128+p, j*128+m] = W*[j*128+m, c*128+p]
    ws_d = nc.declare_dram_parameter("ws", [8, 128, 8, 128], BF16, isOutput=False)
    # Wv row chunks: wv[c] = Wv[c*128:(c+1)*128, :]
    wv_d = nc.declare_dram_parameter("wv", [8, 128, DIM], BF16, isOutput=False)
    out_d = nc.declare_dram_parameter("out", [NQ, DIM], BF16, isOutput=True)
    z_d = nc.declare_dram_parameter("zout", [1, NQ], F32, isOutput=True)

    with tile.TileContext(nc) as tc:
        # Long-lived pool: on-chip intermediates live to the end.
        L = tc.alloc_tile_pool(name="L", bufs=1)
        pm = tc.alloc_tile_pool(name="pm", bufs=1, space="PSUM")
        # Transient input pools, released once consumed (LIFO: t2 first).
        t1 = tc.alloc_tile_pool(name="t1", bufs=1)  # ws slabs (+warm tile)
        t2 = tc.alloc_tile_pool(name="t2", bufs=1)  # wv chunks

        # ---- HAM warm-up: dummy matmuls on a zeroed scratch tile run
        # during the otherwise-idle input-DMA window, flipping the PE clock
        # gate to 8/8 (2.4GHz) before the first real matmul arrives
        # (~3.4us of sustained PE activity required; 8 cold MMs x 427ns).
        wsc = t1.tile([128, 512], BF16, name="warm", tag="warm", bufs=1)
        nc.vector.memset(wsc[:], 0.0)
        # per-partition bias constant -ln2 for the scaled exp (see P4)
        nln2 = L.tile([128, 1], F32, name="nln2", bufs=1)
        nc.vector.memset(nln2[:], -LN2)
        wps = pm.tile([128, 512], F32, name="wps", tag="warm", bufs=1)
        for w in range(N_WARM):
            nc.tensor.matmul(
                wps[:], lhsT=wsc[:, 0:128], rhs=wsc[:],
                start=(w == 0), stop=(w == N_WARM - 1),
            )

        # ---- Input DMAs.  All on the sync queue, in consumption order
        # (yT, wv, ws, xT).  Measured
        # extensively: a lone queue sustains ~166GB/s; adding queues makes
        # each drop to ~100GB/s (shared SDMA engines) and slows the
        # head-critical yT.  Gating bulk transfers behind completion of
        # the critical set (via blocker ops) delays ws/xT too much.  The
        # simple single-queue FIFO is the best measured configuration.
        def dma(out_ap, in_ap):
            nc.sync.dma_start(out=out_ap, in_=in_ap)

        ytr = yT_d.rearrange("(c p) n -> c p n", p=128)
        ws_slabs = [
            t1.tile([128, 8, 128], BF16, name=f"ws{j}", tag="ws", bufs=8)
            for j in range(8)
        ]
        ytc = [
            L.tile([128, NKV_SHARD], BF16, name=f"yt{c}", tag="yt", bufs=8)
            for c in range(8)
        ]
        wvc = [
            t2.tile([128, DIM], BF16, name=f"wv{c}", tag="wv", bufs=8)
            for c in range(8)
        ]
        xtr = xT_d.rearrange("(c p) n -> c p n", p=128)
        xtc = [
            L.tile([128, NQ], BF16, name=f"xt{c}", tag="xt", bufs=8)
            for c in range(8)
        ]
        for c in range(8):
            dma(ytc[c][:], ytr[c])
        for c in range(8):
            dma(wvc[c][:, 0:512], wv_d[c][:, 0:512])
        for c in range(8):
            dma(wvc[c][:, 512:1024], wv_d[c][:, 512:1024])
        for j in range(8):
            dma(ws_slabs[j][:], ws_d[j])
        for c in range(8):
            dma(xtc[c][:], xtr[c])

        # ---- P3: v[nkv, do] = sum_d yT[d, nkv] * Wv[d, do] --------------
        # kv-tiles 0-1 are stored fp8 in DoubleRow-interleaved layout
        # [128, ko=2, do] (element (p,ko,d) = v[kv=ko*128+p, d]) so P7 can
        # contract them in a single double-pumped fp8 matmul.  Partial fp8
        # (2 of 8 kv-tiles) keeps the added error at ~sqrt(1/4)*2.6e-2 =
        # 1.3e-2, inside the 2e-2 gate with margin.
        vtf = L.tile([128, 2, DIM], FP8, name="vtf", tag="vtf", bufs=1)
        vt = [None, None] + [
            L.tile([128, DIM], BF16, name=f"v{i}", tag="v", bufs=6) for i in range(2, 8)
        ]
        for dd in range(2):  # d_out 512-chunk
            for i in range(8):  # nkv 128-tile
                ps = pm.tile([128, 512], F32, name=f"psv{i}_{dd}", tag="mm", bufs=4)
                for c in range(8):
                    nc.tensor.matmul(
                        ps[:],
                        lhsT=ytc[c][:, ds(i * 128, 128)],
                        rhs=wvc[c][:, ds(dd * 512, 512)],
                        start=(c == 0),
                        stop=(c == 7),
                    )
                if i < 2:
                    nc.any.tensor_copy(vtf[:, i, ds(dd * 512, 512)], ps[:])
                else:
                    nc.any.tensor_copy(vt[i][:, ds(dd * 512, 512)], ps[:])
        t2.release()

        # ---- P2: u[d, k] = sum_e W*T[e, d] * yT[e, k] --------------------
        # (u = W* @ y^T, kv-sharded by construction -- replaces the old
        # t = x @ W* whole-query projection that was duplicated per pair.)
        ut = [L.tile([128, NKV_SHARD], BF16, name=f"u{j}", tag="u", bufs=8)
              for j in range(8)]
        for j in range(8):  # d 128-chunk (output rows)
            for kq in range(2):  # kv 512-chunk
                ps = pm.tile([128, 512], F32, name=f"psu{j}_{kq}", tag="mm", bufs=4)
                for c in range(8):  # e chunk (contraction)
                    nc.tensor.matmul(
                        ps[:],
                        lhsT=ws_slabs[j][:, c, :],
                        rhs=ytc[c][:, ds(kq * 512, 512)],
                        start=(c == 0),
                        stop=(c == 7),
                    )
                nc.any.tensor_copy(ut[j][:, ds(kq * 512, 512)], ps[:])
        t1.release()

        # ---- P4: expT[nkv, nq] = 0.5*exp(scores) ------------------------
        # ALL exp tiles carry a global 0.5 scale (bias=-ln2): max score is
        # ~5.5 so raw exp reaches ~244 > fp8e4's 240 (-> Inf on convert);
        # 0.5*exp <= 122 is safe.  The scale cancels exactly in out=num/Z.
        etf = L.tile([128, 2, NQ], FP8, name="etf", tag="etf", bufs=1)
        et = [None, None] + [
            L.tile([128, NQ], BF16, name=f"e{i}", tag="et", bufs=6) for i in range(2, 8)
        ]
        for i in range(8):  # nkv 128-tile
            for q in range(4):  # nq 512-chunk
                ps = pm.tile([128, 512], F32, name=f"pse{i}_{q}", tag="mm", bufs=4)
                for c in range(8):  # d chunk (contraction)
                    nc.tensor.matmul(
                        ps[:],
                        lhsT=ut[c][:, ds(i * 128, 128)],
                        rhs=xtc[c][:, ds(q * 512, 512)],
                        start=(c == 0),
                        stop=(c == 7),
                    )
                dst = etf[:, i, ds(q * 512, 512)] if i < 2 else et[i][:, ds(q * 512, 512)]
                nc.scalar.activation(
                    dst,
                    ps[:],
                    mybir.ActivationFunctionType.Exp,
                    bias=nln2[:],
                )

        # ---- Z add-tree on the (otherwise idle) vector engine: collapse
        # the 8 et tiles to one bf16 [128, NQ]; runs concurrently with the
        # P4/P7 matmul stream (gated only on et readiness).  bf16 so the Z
        # ones-matmuls below stay single-pass; Z error from bf16 sums
        # averages down ~sqrt(128) in the partition reduction.
        t3 = tc.alloc_tile_pool(name="t3", bufs=1)
        s0 = [t3.tile([128, NQ], BF16, name=f"es0_{h}", tag="es", bufs=3) for h in range(2)]
        nc.vector.tensor_add(s0[0][:], etf[:, 0, :], etf[:, 1, :])
        nc.vector.tensor_add(s0[1][:], et[2][:], et[3][:])
        s1 = t3.tile([128, NQ], BF16, name="es1", tag="es2", bufs=2)
        nc.vector.tensor_add(s1[:], s0[0][:], s0[1][:])
        s0b = [t3.tile([128, NQ], BF16, name=f"es0b_{h}", tag="es", bufs=3) for h in range(2)]
        nc.vector.tensor_add(s0b[0][:], et[4][:], et[5][:])
        nc.vector.tensor_add(s0b[1][:], et[6][:], et[7][:])
        s2 = t3.tile([128, NQ], BF16, name="es2", tag="es2", bufs=2)
        nc.vector.tensor_add(s2[:], s0b[0][:], s0b[1][:])
        stot = t3.tile([128, NQ], BF16, name="estot", tag="es", bufs=3)
        nc.vector.tensor_add(stot[:], s1[:], s2[:])
        ones = L.tile([128, 1], BF16, name="ones", bufs=1)
        nc.vector.memset(ones[:], 1.0)

        # ---- P7: out'[nq, do] = sum_nkv expT[nkv,nq] * v[nkv,do] --------
        # The Z ones-matmuls ([1,512] partition reductions of the vector
        # add-tree result) slot in mid-stream at t==8: stot is ready ~4us
        # after P4 ends, long before then, so the PE never stalls and the
        # tail only carries the last out-tile's copy+DMA.
        for t in range(16):  # nq 128-tile
            for dd in range(2):  # d_out 512-chunk
                if t == 15 and dd == 1:
                    # Tail: two N=256 groups so copy+DMA of the first half
                    # overlaps the second half's matmuls, halving the drain.
                    for h in range(2):
                        psh = pm.tile([128, 256], F32, name=f"psoL{h}", tag="mmL", bufs=2)
                        for i in range(2, 8):
                            nc.tensor.matmul(
                                psh[:],
                                lhsT=et[i][:, ds(t * 128, 128)],
                                rhs=vt[i][:, ds(dd * 512 + h * 256, 256)],
                                start=(i == 2),
                                stop=False,
                            )
                        nc.tensor.matmul(
                            psh[:],
                            lhsT=etf[:, :, ds(t * 128, 128)],
                            rhs=vtf[:, :, ds(dd * 512 + h * 256, 256)],
                            start=False,
                            stop=True,
                            perf_mode=mybir.MatmulPerfMode.DoubleRow,
                        )
                        obh = L.tile([128, 256], BF16, name=f"oL{h}", tag="oL", bufs=2)
                        # force DVE: ~190ns vs the scalar ACTIVATE's ~475ns,
                        # and this copy sits on the final-drain critical path
                        nc.vector.tensor_copy(obh[:], psh[:])
                        nc.sync.dma_start(
                            out=out_d[ds(t * 128, 128), ds(dd * 512 + h * 256, 256)],
                            in_=obh[:],
                        )
                    continue
                ps = pm.tile([128, 512], F32, name=f"pso{t}_{dd}", tag="mm", bufs=4)
                for i in range(2, 8):  # nkv contraction, bf16 tiles
                    nc.tensor.matmul(
                        ps[:],
                        lhsT=et[i][:, ds(t * 128, 128)],
                        rhs=vt[i][:, ds(dd * 512, 512)],
                        start=(i == 2),
                        stop=False,
                    )
                # kv 0-255 in one double-pumped fp8 matmul (DoubleRow)
                nc.tensor.matmul(
                    ps[:],
                    lhsT=etf[:, :, ds(t * 128, 128)],
                    rhs=vtf[:, :, ds(dd * 512, 512)],
                    start=False,
                    stop=True,
                    perf_mode=mybir.MatmulPerfMode.DoubleRow,
                )
                ob = L.tile([128, 512], BF16, name=f"o{t}_{dd}", tag="o", bufs=3)
                nc.any.tensor_copy(ob[:], ps[:])
                nc.sync.dma_start(
                    out=out_d[ds(t * 128, 128), ds(dd * 512, 512)], in_=ob[:]
                )
            if t == 8:
                for q in range(4):
                    psz = pm.tile([1, 512], F32, name=f"psz{q}", tag="zr", bufs=1)
                    nc.tensor.matmul(
                        psz[:],
                        lhsT=ones[:],
                        rhs=stot[:, ds(q * 512, 512)],
                        start=True,
                        stop=True,
                    )
                    zrow = L.tile([1, 512], F32, name=f"zrow{q}", tag="zrow", bufs=2)
                    nc.any.tensor_copy(zrow[:], psz[:])
                    nc.sync.dma_start(out=z_d[0:1, ds(q * 512, 512)], in_=zrow[:])
        t3.release()
        pm.release()
        L.release()

    _early_sp_start(nc)
    _split_sync_waits(nc)
    return nc


_NC_CACHE = {}


def _get_nc():
    if "nc" not in _NC_CACHE:
        _NC_CACHE["nc"] = build_attention_nc()
    return _NC_CACHE["nc"]


def make_in_maps(x, y, Wq, Wkv):
    """Host-side sharding + layout prep. Returns in_maps for cores 0-7."""
    scale = DIM ** (-0.5)
    wkv = np.asarray(Wkv, np.float32)
    # W* = scale * Wq @ Wk^T (fp32, once); ship W*^T in bf16 slabs:
    # ws[j, p, c, m] = W*T[c*128+p, j*128+m]
    wstar_t = ((np.asarray(Wq, np.float32) * scale) @ wkv[:, :DIM].T).T
    ws = np.ascontiguousarray(
        wstar_t.astype(NP_BF16).reshape(8, 128, 8, 128).transpose(2, 1, 0, 3)
    )
    wv = np.ascontiguousarray(wkv[:, DIM:].astype(NP_BF16).reshape(8, 128, DIM))

    x = np.asarray(x, np.float32)
    y = np.asarray(y, np.float32)
    in_maps = []
    for core in range(N_CORES):
        b, s = divmod(core, 2)
        xT = np.ascontiguousarray(x[b].T).astype(NP_BF16)
        yT = np.ascontiguousarray(
            y[b, s * NKV_SHARD : (s + 1) * NKV_SHARD, :].T
        ).astype(NP_BF16)
        in_maps.append({"xT": xT, "yT": yT, "ws": ws, "wv": wv})
    return in_maps


def run_sharded(x, y, Wq, Wkv, trace=False, tmpdir=None):
    """Run the SPMD kernel; returns (full_output, BassKernelResults)."""
    nc = _get_nc()
    in_maps = make_in_maps(x, y, Wq, Wkv)
    try:
        res = run_bass_kernel_spmd(
            nc, in_maps, core_ids=list(range(N_CORES)), trace=trace, tmpdir=tmpdir
        )
    except Exception:
        # one retry: transient NRT device states (e.g. a previous crashed
        # load) usually clear on the next attempt
        res = run_bass_kernel_spmd(
            nc, in_maps, core_ids=list(range(N_CORES)), trace=trace, tmpdir=tmpdir
        )
    out = np.empty((B, NQ, DIM), np.float32)
    for b in range(B):
        r0, r1 = res.results[2 * b], res.results[2 * b + 1]
        num = r0["out"].astype(np.float32) + r1["out"].astype(np.float32)
        z = (r0["zout"].astype(np.float32) + r1["zout"].astype(np.float32))[0]
        out[b] = num / z[:, None]
    return out, res


def kernel(x, y, Wq, Wkv):
    out, _ = run_sharded(x, y, Wq, Wkv)
    return out


# revision 46
# speedup vs baseline: 1.0577x; 1.0577x over previous
"""Self-contained Trainium2 Bass kernel for single-head full-dim attention.

Reference computation (fp32 jax):
    q  = x @ Wq                      # [B, Nq, D]
    kv = y @ Wkv                     # [B, Nkv, 2D] -> k, v
    attn = softmax(q * D^-0.5 @ k^T) # [B, Nq, Nkv]
    out  = attn @ v                  # [B, Nq, D]
with B=4, Nq=Nkv=2048, D=1024.

Distribution: data parallel over 8 NeuronCores, shard = (batch b,
kv-half s).  Each core computes the UNNORMALIZED output block
out'_s = exp(S_s) @ v_s for its 1024 keys plus the partial softmax
denominator Z_s; the host combines the halves:
out = (out'_0 + out'_1) / (Z_0 + Z_1).  No collectives.

Algebraic fold + reassociation: scores = x @ (scale*Wq@Wk^T) @ y^T.
The host precomputes W* = scale*Wq@Wk^T once (fp32, free), and the
device associates the product as  x @ (W* @ y^T):
    P2:  u_s = W* @ y_s^T        [D, 1024]   (kv-SHARDED -> no dup!)
    P4:  scoresT_s = u_s^T-contracted with x (via lhsT=u tiles)
Compared to the t = x@W* association (which every core must compute
for ALL 2048 queries, duplicated across the kv-pair), u is per-shard
by construction: per-core MACs drop 7.52e9 -> 6.45e9, the theoretical
no-duplication floor, with zero communication.

Layout trick: everything on-chip is computed transposed
([feature, token]) so the TensorEngine contracts along partitions
without any on-chip transposes.  Matmul operands are bf16 (fp32 PSUM
accumulation), EXCEPT kv-tiles 0-1 of P7's contraction, which are fp8
e4m3 in DoubleRow-interleaved [128, ko=2, n] layout: one double-pumped
DoubleRow matmul replaces two bf16 matmuls per output group (-5.7us).
Partial fp8 (2 of 8 kv-tiles) adds sqrt(1/4)*2.6e-2 ~ 1.3e-2 error;
total 1.40e-2 vs the 2e-2 gate.  All exp tiles carry a global 0.5
scale (activation bias=-ln2) so fp8 conversion cannot overflow (raw
exp reaches ~244 > fp8e4's 240 max -> Inf); the scale cancels in
out = num/Z.  Softmax uses exp without max-subtraction (scores ~
N(0,1) by construction); Z is a bf16 vector-engine add-tree plus 4
bf16 ones-matmuls (f32 ones-matmuls run 2-pass LOW_HIGH on the PE).
Outputs are bf16 (host combines in f32) which halves writeback DMA.

DMA: all input DMAs on the sync queue in consumption order (yT, wv,
ws, xT).  A lone queue sustains ~166GB/s; spreading across engines
makes each queue drop to ~100GB/s (shared SDMA engine pool) and slows
the head-critical yT -- measured, not theorized.
"""

import numpy as np
import ml_dtypes

import concourse.bass as bass
import concourse.mybir as mybir
import concourse.tile as tile
from concourse.bass import ds
from concourse.bass_utils import run_bass_kernel_spmd

DIM = 1024
B = 4
NQ = 2048
NKV = 2048
N_CORES = 8
NKV_SHARD = 1024  # keys per core

BF16 = mybir.dt.bfloat16
F32 = mybir.dt.float32
FP8 = mybir.dt.float8e4
LN2 = 0.6931471805599453
NP_BF16 = ml_dtypes.bfloat16

N_WARM = 20


def _split_sync_waits(nc, max_waits: int = 1):
    """walrus in this toolchain rejects instructions carrying more than one
    sem wait ("Too many sync wait commands").  Hoist extra waits onto
    preceding same-engine NOPs: the engine dispatches in order, so waiting
    just before the instruction is semantically identical (at worst it
    delays issue slightly)."""
    import bass_rust as _bass_rust

    for f in nc.m.functions:
        for bb in f.blocks:
            insts = list(bb.instructions)
            out = []
            changed = False
            for inst in insts:
                si = getattr(inst, "sync_info", None)
                waits = list(si.on_wait) if si is not None and si.on_wait else []
                if len(waits) > max_waits:
                    changed = True
                    extra, keep = waits[:-max_waits], waits[-max_waits:]
                    for k in range(0, len(extra), max_waits):
                        nop = mybir.InstNoOp(
                            name=f"{inst.name}_sw{k}", engine=inst.engine,
                            ins=[], outs=[],
                        )
                        nop.sync_info = _bass_rust.SyncInfo(
                            on_wait=extra[k : k + max_waits], on_update=[]
                        )
                        out.append(nop)
                    si.on_wait = keep
                    inst.sync_info = si
                out.append(inst)
            if changed:
                bb.instructions = out


def _early_sp_start(nc):
    """Release the SP (sync) engine from the TileContext entry barrier in
    the `main` block so input dma_starts issue ~2.3us earlier.

    The entry barrier is held hostage by the PE engine's wait on the
    runtime start event ($E[4], ~3us).  SP's input DMAs have no real
    dependency on the other engines: every consumer of a DMA'd tile
    waits on that DMA's completion semaphore anyway.  Protocol-preserving
    edit: SP's drain keeps its arrival increment (gather still sees 4
    arrivals) but loses its wait; SP's barrier event-sem is emptied (no
    wait, no release consumption); the Pool release add drops 4 -> 3 to
    match the remaining three consumers."""
    import bass_rust as _bass_rust

    main = nc.m.functions[0].blocks[0]
    assert main.name == "main", main.name
    for inst in main.instructions:
        eng = str(inst.engine)
        tn = type(inst).__name__
        si = getattr(inst, "sync_info", None)
        if si is None:
            continue
        waits = list(si.on_wait or [])
        upds = list(si.on_update or [])
        is_barrier_wait = any(
            getattr(w, "ant_name", "").startswith("barrier_") for w in waits
        )
        if eng.endswith("SP") and tn == "InstDrain" and is_barrier_wait:
            inst.sync_info = _bass_rust.SyncInfo(on_wait=[], on_update=upds)
        elif eng.endswith("SP") and tn == "InstEventSemaphore" and is_barrier_wait:
            inst.sync_info = _bass_rust.SyncInfo(on_wait=[], on_update=[])
        elif eng.endswith("Pool") and tn == "InstEventSemaphore" and not waits:
            # the release: add-imm 4 -> 3
            new_upds = []
            for u in upds:
                if u.update_mode == "sem-add-imm" and u.update_value == 4:
                    u = _bass_rust.SyncUpdate(
                        sync_type=u.sync_type, id=u.id, ant_name=u.ant_name,
                        update_mode=u.update_mode, update_value=3,
                        update_reg=u.update_reg,
                    )
                new_upds.append(u)
            inst.sync_info = _bass_rust.SyncInfo(on_wait=[], on_update=new_upds)


def build_attention_nc():
    """Build the per-core Bass graph (identical on all 8 cores)."""
    nc = bass.Bass(enable_partition_id=False)

    # DRAM parameters (per-core shards, host-prepped layouts; all bf16).
    xT_d = nc.declare_dram_parameter("xT", [DIM, NQ], BF16, isOutput=False)
    yT_d = nc.declare_dram_parameter("yT", [DIM, NKV_SHARD], BF16, isOutput=False)
    # W*^T slabs pre-arranged so each DMA is per-partition contiguous:
    # ws[j, p, c, m] = W*T[c*128+p, j*128+m] = W*[j*128+m, c*128+p]
    ws_d = nc.declare_dram_parameter("ws", [8, 128, 8, 128], BF16, isOutput=False)
    # Wv row chunks: wv[c] = Wv[c*128:(c+1)*128, :]
    wv_d = nc.declare_dram_parameter("wv", [8, 128, DIM], BF16, isOutput=False)
    out_d = nc.declare_dram_parameter("out", [NQ, DIM], BF16, isOutput=True)
    z_d = nc.declare_dram_parameter("zout", [1, NQ], F32, isOutput=True)

    with tile.TileContext(nc) as tc:
        # Long-lived pool: on-chip intermediates live to the end.
        L = tc.alloc_tile_pool(name="L", bufs=1)
        pm = tc.alloc_tile_pool(name="pm", bufs=1, space="PSUM")
        # Transient input pools, released once consumed (LIFO: t2 first).
        t1 = tc.alloc_tile_pool(name="t1", bufs=1)  # ws slabs (+warm tile)
        t2 = tc.alloc_tile_pool(name="t2", bufs=1)  # wv chunks

        # ---- HAM warm-up: dummy matmuls on a zeroed scratch tile run
        # during the otherwise-idle input-DMA window, flipping the PE clock
        # gate to 8/8 (2.4GHz) before the first real matmul arrives
        # (~3.4us of sustained PE activity required; 8 cold MMs x 427ns).
        wsc = t1.tile([128, 512], BF16, name="warm", tag="warm", bufs=1)
        nc.vector.memset(wsc[:], 0.0)
        # per-partition bias constant -ln2 for the scaled exp (see P4)
        nln2 = L.tile([128, 1], F32, name="nln2", bufs=1)
        nc.vector.memset(nln2[:], -LN2)
        wps = pm.tile([128, 512], F32, name="wps", tag="warm", bufs=1)
        for w in range(N_WARM):
            nc.tensor.matmul(
                wps[:], lhsT=wsc[:, 0:128], rhs=wsc[:],
                start=(w == 0), stop=(w == N_WARM - 1),
            )

        # ---- Input DMAs.  All on the sync queue, in consumption order
        # (yT, wv, ws, xT).  Measured
        # extensively: a lone queue sustains ~166GB/s; adding queues makes
        # each drop to ~100GB/s (shared SDMA engines) and slows the
        # head-critical yT.  Gating bulk transfers behind completion of
        # the critical set (via blocker ops) delays ws/xT too much.  The
        # simple single-queue FIFO is the best measured configuration.
        def dma(out_ap, in_ap):
            nc.sync.dma_start(out=out_ap, in_=in_ap)

        ytr = yT_d.rearrange("(c p) n -> c p n", p=128)
        ws_slabs = [
            t1.tile([128, 8, 128], BF16, name=f"ws{j}", tag="ws", bufs=8)
            for j in range(8)
        ]
        ytc = [
            L.tile([128, NKV_SHARD], BF16, name=f"yt{c}", tag="yt", bufs=8)
            for c in range(8)
        ]
        wvc = [
            t2.tile([128, DIM], BF16, name=f"wv{c}", tag="wv", bufs=8)
            for c in range(8)
        ]
        xtr = xT_d.rearrange("(c p) n -> c p n", p=128)
        xtc = [
            L.tile([128, NQ], BF16, name=f"xt{c}", tag="xt", bufs=8)
            for c in range(8)
        ]
        for c in range(8):
            dma(ytc[c][:], ytr[c])
        for c in range(8):
            dma(wvc[c][:, 0:512], wv_d[c][:, 0:512])
        for c in range(8):
            dma(wvc[c][:, 512:1024], wv_d[c][:, 512:1024])
        for j in range(8):
            dma(ws_slabs[j][:], ws_d[j])
        for c in range(8):
            dma(xtc[c][:], xtr[c])

        # ---- P3: v[nkv, do] = sum_d yT[d, nkv] * Wv[d, do] --------------
        # kv-tiles 0-1 are stored fp8 in DoubleRow-interleaved layout
        # [128, ko=2, do] (element (p,ko,d) = v[kv=ko*128+p, d]) so P7 can
        # contract them in a single double-pumped fp8 matmul.  Partial fp8
        # (2 of 8 kv-tiles) keeps the added error at ~sqrt(1/4)*2.6e-2 =
        # 1.3e-2, inside the 2e-2 gate with margin.
        vtf = L.tile([128, 2, DIM], FP8, name="vtf", tag="vtf", bufs=1)
        vtf2 = L.tile([128, 2, DIM], FP8, name="vtf2", tag="vtf2", bufs=1)
        vt = [None, None, None, None] + [
            L.tile([128, DIM], BF16, name=f"v{i}", tag="v", bufs=4) for i in range(4, 8)
        ]
        for dd in range(2):  # d_out 512-chunk
            for i in range(8):  # nkv 128-tile
                ps = pm.tile([128, 512], F32, name=f"psv{i}_{dd}", tag="mm", bufs=4)
                for c in range(8):
                    nc.tensor.matmul(
                        ps[:],
                        lhsT=ytc[c][:, ds(i * 128, 128)],
                        rhs=wvc[c][:, ds(dd * 512, 512)],
                        start=(c == 0),
                        stop=(c == 7),
                    )
                if i < 2:
                    nc.any.tensor_copy(vtf[:, i, ds(dd * 512, 512)], ps[:])
                elif i < 4:
                    nc.any.tensor_copy(vtf2[:, i - 2, ds(dd * 512, 512)], ps[:])
                else:
                    nc.any.tensor_copy(vt[i][:, ds(dd * 512, 512)], ps[:])
        t2.release()

        # ---- P2: u[d, k] = sum_e W*T[e, d] * yT[e, k] --------------------
        # (u = W* @ y^T, kv-sharded by construction -- replaces the old
        # t = x @ W* whole-query projection that was duplicated per pair.)
        ut = [L.tile([128, NKV_SHARD], BF16, name=f"u{j}", tag="u", bufs=8)
              for j in range(8)]
        for j in range(8):  # d 128-chunk (output rows)
            for kq in range(2):  # kv 512-chunk
                ps = pm.tile([128, 512], F32, name=f"psu{j}_{kq}", tag="mm", bufs=4)
                for c in range(8):  # e chunk (contraction)
                    nc.tensor.matmul(
                        ps[:],
                        lhsT=ws_slabs[j][:, c, :],
                        rhs=ytc[c][:, ds(kq * 512, 512)],
                        start=(c == 0),
                        stop=(c == 7),
                    )
                nc.any.tensor_copy(ut[j][:, ds(kq * 512, 512)], ps[:])
        t1.release()

        # ---- P4: expT[nkv, nq] = 0.5*exp(scores) ------------------------
        # ALL exp tiles carry a global 0.5 scale (bias=-ln2): max score is
        # ~5.5 so raw exp reaches ~244 > fp8e4's 240 (-> Inf on convert);
        # 0.5*exp <= 122 is safe.  The scale cancels exactly in out=num/Z.
        etf = L.tile([128, 2, NQ], FP8, name="etf", tag="etf", bufs=1)
        etf2 = L.tile([128, 2, NQ], FP8, name="etf2", tag="etf2", bufs=1)
        et = [None, None, None, None] + [
            L.tile([128, NQ], BF16, name=f"e{i}", tag="et", bufs=4) for i in range(4, 8)
        ]
        for i in range(8):  # nkv 128-tile
            for q in range(4):  # nq 512-chunk
                ps = pm.tile([128, 512], F32, name=f"pse{i}_{q}", tag="mm", bufs=4)
                for c in range(8):  # d chunk (contraction)
                    nc.tensor.matmul(
                        ps[:],
                        lhsT=ut[c][:, ds(i * 128, 128)],
                        rhs=xtc[c][:, ds(q * 512, 512)],
                        start=(c == 0),
                        stop=(c == 7),
                    )
                dst = (etf[:, i, ds(q * 512, 512)] if i < 2
                       else etf2[:, i - 2, ds(q * 512, 512)] if i < 4
                       else et[i][:, ds(q * 512, 512)])
                nc.scalar.activation(
                    dst,
                    ps[:],
                    mybir.ActivationFunctionType.Exp,
                    bias=nln2[:],
                )

        # ---- Z add-tree on the (otherwise idle) vector engine: collapse
        # the 8 et tiles to one bf16 [128, NQ]; runs concurrently with the
        # P4/P7 matmul stream (gated only on et readiness).  bf16 so the Z
        # ones-matmuls below stay single-pass; Z error from bf16 sums
        # averages down ~sqrt(128) in the partition reduction.
        t3 = tc.alloc_tile_pool(name="t3", bufs=1)
        s0 = [t3.tile([128, NQ], BF16, name=f"es0_{h}", tag="es", bufs=3) for h in range(2)]
        nc.vector.tensor_add(s0[0][:], etf[:, 0, :], etf[:, 1, :])
        nc.vector.tensor_add(s0[1][:], etf2[:, 0, :], etf2[:, 1, :])
        s1 = t3.tile([128, NQ], BF16, name="es1", tag="es2", bufs=2)
        nc.vector.tensor_add(s1[:], s0[0][:], s0[1][:])
        s0b = [t3.tile([128, NQ], BF16, name=f"es0b_{h}", tag="es", bufs=3) for h in range(2)]
        nc.vector.tensor_add(s0b[0][:], et[4][:], et[5][:])
        nc.vector.tensor_add(s0b[1][:], et[6][:], et[7][:])
        s2 = t3.tile([128, NQ], BF16, name="es2", tag="es2", bufs=2)
        nc.vector.tensor_add(s2[:], s0b[0][:], s0b[1][:])
        stot = t3.tile([128, NQ], BF16, name="estot", tag="es", bufs=3)
        nc.vector.tensor_add(stot[:], s1[:], s2[:])
        ones = L.tile([128, 1], BF16, name="ones", bufs=1)
        nc.vector.memset(ones[:], 1.0)

        # ---- P7: out'[nq, do] = sum_nkv expT[nkv,nq] * v[nkv,do] --------
        # The Z ones-matmuls ([1,512] partition reductions of the vector
        # add-tree result) slot in mid-stream at t==8: stot is ready ~4us
        # after P4 ends, long before then, so the PE never stalls and the
        # tail only carries the last out-tile's copy+DMA.
        for t in range(16):  # nq 128-tile
            for dd in range(2):  # d_out 512-chunk
                if t == 15 and dd == 1:
                    # Tail: two N=256 groups so copy+DMA of the first half
                    # overlaps the second half's matmuls, halving the drain.
                    for h in range(2):
                        psh = pm.tile([128, 256], F32, name=f"psoL{h}", tag="mmL", bufs=2)
                        for i in range(4, 8):
                            nc.tensor.matmul(
                                psh[:],
                                lhsT=et[i][:, ds(t * 128, 128)],
                                rhs=vt[i][:, ds(dd * 512 + h * 256, 256)],
                                start=(i == 4),
                                stop=False,
                            )
                        nc.tensor.matmul(
                            psh[:],
                            lhsT=etf[:, :, ds(t * 128, 128)],
                            rhs=vtf[:, :, ds(dd * 512 + h * 256, 256)],
                            start=False,
                            stop=False,
                            perf_mode=mybir.MatmulPerfMode.DoubleRow,
                        )
                        nc.tensor.matmul(
                            psh[:],
                            lhsT=etf2[:, :, ds(t * 128, 128)],
                            rhs=vtf2[:, :, ds(dd * 512 + h * 256, 256)],
                            start=False,
                            stop=True,
                            perf_mode=mybir.MatmulPerfMode.DoubleRow,
                        )
                        obh = L.tile([128, 256], BF16, name=f"oL{h}", tag="oL", bufs=2)
                        # force DVE: ~190ns vs the scalar ACTIVATE's ~475ns,
                        # and this copy sits on the final-drain critical path
                        nc.vector.tensor_copy(obh[:], psh[:])
                        nc.sync.dma_start(
                            out=out_d[ds(t * 128, 128), ds(dd * 512 + h * 256, 256)],
                            in_=obh[:],
                        )
                    continue
                ps = pm.tile([128, 512], F32, name=f"pso{t}_{dd}", tag="mm", bufs=4)
                for i in range(4, 8):  # nkv contraction, bf16 tiles
                    nc.tensor.matmul(
                        ps[:],
                        lhsT=et[i][:, ds(t * 128, 128)],
                        rhs=vt[i][:, ds(dd * 512, 512)],
                        start=(i == 4),
                        stop=False,
                    )
                # kv 0-511 in two double-pumped fp8 matmuls (DoubleRow)
                nc.tensor.matmul(
                    ps[:],
                    lhsT=etf[:, :, ds(t * 128, 128)],
                    rhs=vtf[:, :, ds(dd * 512, 512)],
                    start=False,
                    stop=False,
                    perf_mode=mybir.MatmulPerfMode.DoubleRow,
                )
                nc.tensor.matmul(
                    ps[:],
                    lhsT=etf2[:, :, ds(t * 128, 128)],
                    rhs=vtf2[:, :, ds(dd * 512, 512)],
                    start=False,
                    stop=True,
                    perf_mode=mybir.MatmulPerfMode.DoubleRow,
                )
                ob = L.tile([128, 512], BF16, name=f"o{t}_{dd}", tag="o", bufs=3)
                nc.any.tensor_copy(ob[:], ps[:])
                nc.sync.dma_start(
                    out=out_d[ds(t * 128, 128), ds(dd * 512, 512)], in_=ob[:]
                )
            if t == 8:
                for q in range(4):
                    psz = pm.tile([1, 512], F32, name=f"psz{q}", tag="zr", bufs=1)
                    nc.tensor.matmul(
                        psz[:],
                        lhsT=ones[:],
                        rhs=stot[:, ds(q * 512, 512)],
                        start=True,
                        stop=True,
                    )
                    zrow = L.tile([1, 512], F32, name=f"zrow{q}", tag="zrow", bufs=2)
                    nc.any.tensor_copy(zrow[:], psz[:])
                    nc.sync.dma_start(out=z_d[0:1, ds(q * 512, 512)], in_=zrow[:])
        t3.release()
        pm.release()
        L.release()

    _early_sp_start(nc)
    _split_sync_waits(nc)
    return nc


_NC_CACHE = {}


def _get_nc():
    if "nc" not in _NC_CACHE:
        _NC_CACHE["nc"] = build_attention_nc()
    return _NC_CACHE["nc"]


def make_in_maps(x, y, Wq, Wkv):
    """Host-side sharding + layout prep. Returns in_maps for cores 0-7."""
    scale = DIM ** (-0.5)
    wkv = np.asarray(Wkv, np.float32)
    # W* = scale * Wq @ Wk^T (fp32, once); ship W*^T in bf16 slabs:
    # ws[j, p, c, m] = W*T[c*128+p, j*128+m]
    wstar_t = ((np.asarray(Wq, np.float32) * scale) @ wkv[:, :DIM].T).T
    ws = np.ascontiguousarray(
        wstar_t.astype(NP_BF16).reshape(8, 128, 8, 128).transpose(2, 1, 0, 3)
    )
    wv = np.ascontiguousarray(wkv[:, DIM:].astype(NP_BF16).reshape(8, 128, DIM))

    x = np.asarray(x, np.float32)
    y = np.asarray(y, np.float32)
    in_maps = []
    for core in range(N_CORES):
        b, s = divmod(core, 2)
        xT = np.ascontiguousarray(x[b].T).astype(NP_BF16)
        yT = np.ascontiguousarray(
            y[b, s * NKV_SHARD : (s + 1) * NKV_SHARD, :].T
        ).astype(NP_BF16)
        in_maps.append({"xT": xT, "yT": yT, "ws": ws, "wv": wv})
    return in_maps


def run_sharded(x, y, Wq, Wkv, trace=False, tmpdir=None):
    """Run the SPMD kernel; returns (full_output, BassKernelResults)."""
    nc = _get_nc()
    in_maps = make_in_maps(x, y, Wq, Wkv)
    try:
        res = run_bass_kernel_spmd(
            nc, in_maps, core_ids=list(range(N_CORES)), trace=trace, tmpdir=tmpdir
        )
    except Exception:
        # one retry: transient NRT device states (e.g. a previous crashed
        # load) usually clear on the next attempt
        res = run_bass_kernel_spmd(
            nc, in_maps, core_ids=list(range(N_CORES)), trace=trace, tmpdir=tmpdir
        )
    out = np.empty((B, NQ, DIM), np.float32)
    for b in range(B):
        r0, r1 = res.results[2 * b], res.results[2 * b + 1]
        num = r0["out"].astype(np.float32) + r1["out"].astype(np.float32)
        z = (r0["zout"].astype(np.float32) + r1["zout"].astype(np.float32))[0]
        out[b] = num / z[:, None]
    return out, res


def kernel(x, y, Wq, Wkv):
    out, _ = run_sharded(x, y, Wq, Wkv)
    return out
